# revision 24
# baseline (speedup 1.0000x reference)
import re
import sys
from contextlib import ExitStack

import numpy as np

try:
    import concourse  # noqa
except ImportError:
    sys.path.insert(0, "/opt/trn_rl_repo")

import ml_dtypes
import concourse.bass as bass  # noqa
import concourse.dve_ops as dve_ops
import concourse.tile as tile
from concourse import mybir
from concourse.bass_utils import run_bass_kernel_spmd
from concourse.dve_ops import DveOp
from concourse.dve_spec import C0, C1, Spec, Src0, Src1
from concourse.dve_table_gen import dve_ver_for
from concourse.bacc import Bacc

N_CORES = 8
B = 8192
BC = B // N_CORES  # 1024 batch per core
D_IN = 784
KT = 7  # 784 -> 7 k-tiles of 128 (row 784 = ones carries the bias)
D_IN_PAD = KT * 128  # 896
D_H = 1000
HT = 8  # 1000 -> 8 h-tiles of 128
D_H_PAD = HT * 128  # 1024
D_OUT = 10
T = 25
BETA = 0.95
THR = 1.0
NQ = 4  # batch quarters per core
QW = BC // NQ  # 256
NTERMS = 3  # exact bf16 decomposition of W2
MW = NTERMS * D_OUT  # 30: term ti at columns 10*ti..10*ti+9
NE = (T + 7) // 8  # 4 blocks of up-to-8 timesteps packed into one psum tile

LAST_EXEC_NS = None
TRACE = False

_CACHE = {}


def _install_ntff_hook():
    try:
        import antenv.axon_hooks  # noqa

        return
    except ImportError:
        pass
    try:
        import types

        import antenv

        mod = types.ModuleType("antenv.axon_hooks")
        mod._hook = None

        def set_axon_ntff_profile_hook(h):
            mod._hook = h

        def get_axon_ntff_profile_hook():
            return mod._hook

        mod.set_axon_ntff_profile_hook = set_axon_ntff_profile_hook
        mod.get_axon_ntff_profile_hook = get_axon_ntff_profile_hook
        sys.modules["antenv.axon_hooks"] = mod
        antenv.axon_hooks = mod
        try:
            from trn_agent_boot.trn_boot import _ntff_profile_via_ctypes

            hook = _ntff_profile_via_ctypes("/opt/axon/libaxon_pjrt.so")
            if hook is not None:
                mod._hook = hook
        except Exception:
            pass
    except Exception:
        pass


def _register_memupd():
    for op in dve_ops.OPS:
        if op.name == "SNN_MEMUPD":
            return op
    spec = Spec(
        body=Src0 * C0 + Src1 - (Src0 > C1),
        reference=lambda in0, in1, s0, s1, imm2: in0 * s0
        + in1
        - (in0 > s1).astype(np.float32),
    )
    op = DveOp("SNN_MEMUPD", spec, subdim=False, uops_sha={})
    dve_ops.OPS.append(op)
    dve_ops.CUSTOM_DVE_SPECS[op.name] = op.spec
    dve_ops._SUB_OPCODE_FOR_NAME[op.name] = (
        dve_ops._CUSTOM_DVE_ROW_BASE + len(dve_ops.OPS) - 1
    )
    ver = dve_ver_for("TRN2")
    try:
        op.compile(ver)
    except ValueError as e:
        m = re.search(r'uops_sha\["%s"\]="([0-9a-f]+)"' % ver, str(e))
        if not m:
            raise
        op.uops_sha[ver] = m.group(1)
        op.compile(ver)
    return op


def _build():
    MEMUPD = _register_memupd()
    nc = Bacc()
    f32 = mybir.dt.float32
    bf16 = mybir.dt.bfloat16
    AF = mybir.ActivationFunctionType

    xT_d = nc.declare_dram_parameter("xT", [KT, 128, BC], f32, isOutput=False)
    w1T_d = nc.declare_dram_parameter("w1T", [KT, 128, D_H_PAD], f32, isOutput=False)
    w2p_d = nc.declare_dram_parameter("w2p", [128, HT, MW], bf16, isOutput=False)
    evT_d = nc.declare_dram_parameter("evT", [NQ, NE, 128, 2 * QW], f32, isOutput=True)

    with tile.TileContext(nc) as tc, ExitStack() as ctx:
        pool = ctx.enter_context(tc.tile_pool(name="sb", bufs=1))
        ppool = ctx.enter_context(tc.tile_pool(name="ps", bufs=1, space="PSUM"))

        xsb = [pool.tile([128, KT, QW], f32, name=f"x_{i}") for i in range(2)]
        w1sb = pool.tile([128, KT, D_H_PAD], f32)
        cur1q = [pool.tile([128, HT, QW], f32, name=f"cur1_{i}") for i in range(2)]
        mem = [pool.tile([128, HT, QW], f32, name=f"mem_{i}") for i in range(6)]
        # sgn holds a PAIR of timesteps side by side so fc2 runs N=512 matmuls
        sgn = [pool.tile([128, HT, 2, QW], bf16, name=f"sgn_{i}") for i in range(4)]
        w2sb = pool.tile([128, HT, MW], bf16)
        evsb = [pool.tile([128, 2 * QW], f32, name=f"ev_{i}") for i in range(2)]
        negone = pool.tile([128, 1], f32)
        zero = pool.tile([128, 1], f32)

        # fc1 psum: 4 h-tiles of one quarter per tile (2 banks), ping-pong
        p1 = [ppool.tile([128, 4, QW], f32, name=f"p1_{i}") for i in range(2)]
        # fc2 psum: 4 step-PAIRS packed at partition col-groups 32*j (8 steps)
        p2 = [ppool.tile([128, 2 * QW], f32, name=f"p2_{i}") for i in range(2)]

        nc.gpsimd.memset(negone[:], -1.0)
        nc.gpsimd.memset(zero[:], 0.0)

        # Input DMAs, ordered so fc1(q0) can start as early as possible.
        # x streams through 2 rotating quarter buffers; x(q) is DMA'd during
        # quarter q-1 (emitted with the drip).
        for k in range(KT):
            nc.sync.dma_start(w1sb[:, k, 0:512], w1T_d[k, :, 0:512])
            nc.scalar.dma_start(xsb[0][:, k, :], xT_d[k, :, 0:QW])
        nc.sync.dma_start(w2sb[:], w2p_d[:])
        for k in range(KT):
            nc.sync.dma_start(w1sb[:, k, 512:1024], w1T_d[k, :, 512:1024])

        def fc1_thunks(q, copies_on_vector=False):
            """fc1 for quarter q: 56 mm thunks, then the 2 copy thunks last."""
            qs = slice(q * QW, (q + 1) * QW)
            xq = xsb[q % 2]
            mms = []
            cps = []
            for g in range(2):
                par = (2 * q + g) % 2

                for h4 in range(4):
                    h = 4 * g + h4
                    for k in range(KT):
                        def mm(h=h, h4=h4, k=k, par=par):
                            nc.tensor.matmul(
                                p1[par][:, h4, :],
                                w1sb[:, k, 128 * h : 128 * (h + 1)],
                                xq[:, k, :],
                                start=(k == 0),
                                stop=(k == KT - 1),
                            )
                        mms.append(mm)

                def cp(g=g, par=par):
                    dst = cur1q[q % 2][:, 4 * g : 4 * (g + 1), :]
                    if copies_on_vector:
                        nc.vector.tensor_copy(dst, p1[par][:])
                    else:
                        nc.scalar.activation(dst, p1[par][:], AF.Identity, bias=zero[:])
                cps.append(cp)
            # interleave: [g0 mms, cp_g0, g1 mms, cp_g1]
            return mms[:28] + [cps[0]] + mms[28:] + [cps[1]], cps

        # Head: fc1(q0) with g0/g1 interleaved into the vector/scalar queues so
        # the first memupd+sign can start after only the first 4 h-tiles; g0
        # then runs GA steps ahead while fc1(q0,g1) finishes on the PE.
        q0thunks, _ = fc1_thunks(0, copies_on_vector=True)
        for th in q0thunks[:29]:  # g0 mms + cur1-g0 copy (vector)
            th()
        for th in q0thunks[29:57]:  # g1 mms
            th()
        g0, g1 = slice(0, 4), slice(4, HT)

        def memupd_g(t, g, cq):
            in0 = cq[:, g, :] if t == 1 else mem[(t - 1) % 6][:, g, :]
            nc.vector._custom_dve(
                MEMUPD,
                out=mem[t % 6][:, g, :],
                in0=in0,
                in1=cq[:, g, :],
                s0=BETA,
                s1=THR,
            )

        def sign_g(t, g, cq):
            msrc = cq[:, g, :] if t == 0 else mem[t % 6][:, g, :]
            nc.scalar.activation(
                sgn[(t // 2) % 4][:, g, t % 2, :], msrc, AF.Sign, bias=negone[:]
            )

        def fc2_emit(q, t):
            sp = (t // 2) % 4
            j = (t // 2) % 4
            epar = (t // 8) % 2
            pair = t % 2 == 1
            for h in range(HT):
                rhs = sgn[sp][:, h, :, :] if pair else sgn[sp][:, h, 0, :]
                out_ap = (
                    p2[epar][32 * j : 32 * j + MW, :]
                    if pair
                    else p2[epar][32 * j : 32 * j + MW, 0:QW]
                )
                nc.tensor.matmul(
                    out_ap,
                    w2sb[:, h, :],
                    rhs,
                    start=(h == 0),
                    stop=(h == HT - 1),
                    tile_position=(0, 32 * j),
                )

        pending_ev = [None]

        def flush_ev():
            if pending_ev[0] is not None:
                pending_ev[0]()
                pending_ev[0] = None

        GA = 7
        cq0 = cur1q[0]
        sign_g(0, g0, cq0)
        for t in range(1, GA + 1):
            memupd_g(t, g0, cq0)
            sign_g(t, g0, cq0)
        q0thunks[57]()  # vector: cur1 g1 copy
        sign_g(0, g1, cq0)
        for t in range(1, GA + 1):
            memupd_g(t, g1, cq0)
            sign_g(t, g1, cq0)
            if t % 2 == 1:
                fc2_emit(0, t)
        for k in range(KT):
            nc.sync.dma_start(xsb[1][:, k, :], xT_d[k, :, QW : 2 * QW])

        def ev0():
            nc.scalar.activation(evsb[0][:], p2[0][:], AF.Identity, bias=zero[:])
            nc.sync.dma_start(evT_d[0, 0, :, :], evsb[0][:])

        pending_ev[0] = ev0

        for q in range(NQ):
            qs = slice(q * QW, (q + 1) * QW)
            if q + 1 < NQ:
                drip, _ = fc1_thunks(q + 1)
                if q + 2 < NQ:
                    qn = q + 2

                    def xdma(qn=qn):
                        for k in range(KT):
                            nc.sync.dma_start(
                                xsb[qn % 2][:, k, :],
                                xT_d[k, :, qn * QW : (qn + 1) * QW],
                            )
                    drip = drip + [xdma]
            else:
                drip = []
            emitted = 0
            for t in range(8 if q == 0 else 0, T):
                par = t % 6
                cq = cur1q[q % 2]
                if t > 0:
                    in0 = cq[:, :, :] if t == 1 else mem[(t - 1) % 6][:, :, :]
                    nc.vector._custom_dve(
                        MEMUPD,
                        out=mem[par][:, :, :],
                        in0=in0,
                        in1=cq[:, :, :],
                        s0=BETA,
                        s1=THR,
                    )
                nc.scalar.activation(
                    sgn[(t // 2) % 4][:, :, t % 2, :],
                    cq[:, :, :] if t == 0 else mem[par][:, :, :],
                    AF.Sign,
                    bias=negone[:],
                )
                if t % 8 == 3:
                    flush_ev()

                # fc2 on step-pairs (and the lone final step)
                if t % 2 == 1 or t == T - 1:
                    fc2_emit(q, t)

                # keep the PE HAM-warm through the drip-free final quarter
                if q == NQ - 1 and t >= 12:
                    for _ in range(2):
                        nc.tensor.matmul(
                            p1[0][0:MW, 0, 0:MW],
                            w2sb[:, 0, :],
                            w2sb[:, 1, 0:MW],
                            start=True,
                            stop=True,
                        )

                if drip:
                    target = min(len(drip), (t + 1) * len(drip) // 21 + 1)
                    while emitted < target:
                        drip[emitted]()
                        emitted += 1

                if t % 8 == 7 or t == T - 1:
                    e = t // 8
                    evp = e % 2
                    ep = (t // 8) % 2

                    def ev(q=q, e=e, evp=evp, ep=ep):
                        nc.scalar.activation(
                            evsb[evp][:], p2[ep][:], AF.Identity, bias=zero[:]
                        )
                        nc.sync.dma_start(evT_d[q, e, :, :], evsb[evp][:])

                    flush_ev()
                    pending_ev[0] = ev
        flush_ev()

    nc.finalize()
    return nc


def _prep_shared(W1, b1, W2, b2):
    bf = ml_dtypes.bfloat16
    w1T = np.zeros((KT * 128, D_H_PAD), np.float32)
    w1T[:D_IN, :D_H] = W1.T
    w1T[D_IN, :D_H] = b1  # ones-row trick: bias folded into the contraction
    w1T = np.ascontiguousarray(w1T.reshape(KT, 128, D_H_PAD))

    w2pad = np.zeros((D_OUT, D_H_PAD), np.float32)
    w2pad[:, :D_H] = W2
    terms = []
    r = w2pad.copy()
    for _ in range(NTERMS):
        tb = r.astype(bf)
        terms.append(tb)
        r = r - tb.astype(np.float32)

    w2p = np.zeros((128, HT, MW), bf)
    for h in range(HT):
        for ti, tb in enumerate(terms):
            half = (0.5 * tb[:, 128 * h : 128 * (h + 1)].astype(np.float32)).astype(bf)
            w2p[:, h, D_OUT * ti : D_OUT * (ti + 1)] = half.T

    # spk = (sgn+1)/2 so W2@spk = 0.5*W2@sgn + 0.5*sum(W2); fold shift into v.
    v = (b2.astype(np.float64) + 0.5 * w2pad.astype(np.float64).sum(axis=1)).astype(
        np.float32
    )
    return w1T, w2p, v


def kernel(**inputs):
    global LAST_EXEC_NS
    x = np.ascontiguousarray(np.asarray(inputs["x"], dtype=np.float32))
    W1 = np.asarray(inputs["W1"], dtype=np.float32)
    b1 = np.asarray(inputs["b1"], dtype=np.float32)
    W2 = np.asarray(inputs["W2"], dtype=np.float32)
    b2 = np.asarray(inputs["b2"], dtype=np.float32)

    if "nc" not in _CACHE:
        _CACHE["nc"] = _build()
    nc = _CACHE["nc"]

    w1T, w2p, v = _prep_shared(W1, b1, W2, b2)

    in_maps = []
    for c in range(N_CORES):
        xc = x[c * BC : (c + 1) * BC]  # [BC, 784]
        xT = np.zeros((KT * 128, BC), np.float32)
        xT[:D_IN] = xc.T
        xT[D_IN] = 1.0  # ones-row carries b1
        in_maps.append(
            {
                "xT": np.ascontiguousarray(xT.reshape(KT, 128, BC)),
                "w1T": w1T,
                "w2p": w2p,
            }
        )

    if TRACE:
        _install_ntff_hook()
    br = run_bass_kernel_spmd(nc, in_maps, list(range(N_CORES)), trace=TRACE)
    LAST_EXEC_NS = br.exec_time_ns

    # cur2 = 0.5*W2@sgn + v, summed over the 3 bf16 terms on host.
    X = np.empty((T, B, D_OUT), np.float32)
    for c in range(N_CORES):
        evT = br.results[c]["evT"]  # [NQ, NE, 128, 2*QW]
        for q in range(NQ):
            for e in range(NE):
                for j in range(4):
                    for s in range(2):
                        t = 8 * e + 2 * j + s
                        if t >= T:
                            break
                        rows = evT[
                            q, e, 32 * j : 32 * j + MW, s * QW : (s + 1) * QW
                        ]  # [30, QW]
                        ssum = (
                            rows[0:D_OUT]
                            + rows[D_OUT : 2 * D_OUT]
                            + rows[2 * D_OUT :]
                        )
                        b0 = c * BC + q * QW
                        X[t, b0 : b0 + QW, :] = ssum.T

    X += v

    mem2_rec = np.empty((T, B, D_OUT), np.float32)
    beta = np.float32(BETA)
    m2 = X[0]
    mem2_rec[0] = m2
    for t in range(1, T):
        spk = (m2 > THR).astype(np.float32)
        m2 = beta * m2 + X[t] - spk
        mem2_rec[t] = m2
    spk2_rec = (mem2_rec > THR).astype(np.float32)
    return spk2_rec, mem2_rec


# revision 25
# speedup vs baseline: 1.1484x; 1.1484x over previous
import re
import sys
from contextlib import ExitStack

import numpy as np

try:
    import concourse  # noqa
except ImportError:
    sys.path.insert(0, "/opt/trn_rl_repo")

import ml_dtypes
import concourse.bass as bass  # noqa
import concourse.dve_ops as dve_ops
import concourse.tile as tile
from concourse import mybir
from concourse.bass_utils import run_bass_kernel_spmd
from concourse.dve_ops import DveOp
from concourse.dve_spec import C0, C1, Spec, Src0, Src1
from concourse.dve_table_gen import dve_ver_for
from concourse.bacc import Bacc

N_CORES = 8
B = 8192
BC = B // N_CORES  # 1024 batch per core
D_IN = 784
KT = 7  # 784 -> 7 k-tiles of 128 (row 784 = ones carries the bias)
D_IN_PAD = KT * 128  # 896
D_H = 1000
HT = 8  # 1000 -> 8 h-tiles of 128
D_H_PAD = HT * 128  # 1024
D_OUT = 10
T = 25
BETA = 0.95
THR = 1.0
NQ = 4  # batch quarters per core
QW = BC // NQ  # 256
NTERMS = 3  # exact bf16 decomposition of W2
MW = NTERMS * D_OUT  # 30: term ti at columns 10*ti..10*ti+9
NE = (T + 7) // 8  # 4 blocks of up-to-8 timesteps packed into one psum tile

LAST_EXEC_NS = None
TRACE = False

_CACHE = {}


def _install_ntff_hook():
    try:
        import antenv.axon_hooks  # noqa

        return
    except ImportError:
        pass
    try:
        import types

        import antenv

        mod = types.ModuleType("antenv.axon_hooks")
        mod._hook = None

        def set_axon_ntff_profile_hook(h):
            mod._hook = h

        def get_axon_ntff_profile_hook():
            return mod._hook

        mod.set_axon_ntff_profile_hook = set_axon_ntff_profile_hook
        mod.get_axon_ntff_profile_hook = get_axon_ntff_profile_hook
        sys.modules["antenv.axon_hooks"] = mod
        antenv.axon_hooks = mod
        try:
            from trn_agent_boot.trn_boot import _ntff_profile_via_ctypes

            hook = _ntff_profile_via_ctypes("/opt/axon/libaxon_pjrt.so")
            if hook is not None:
                mod._hook = hook
        except Exception:
            pass
    except Exception:
        pass


def _register_memupd():
    for op in dve_ops.OPS:
        if op.name == "SNN_MEMUPD":
            return op
    spec = Spec(
        body=Src0 * C0 + Src1 - (Src0 > C1),
        reference=lambda in0, in1, s0, s1, imm2: in0 * s0
        + in1
        - (in0 > s1).astype(np.float32),
    )
    op = DveOp("SNN_MEMUPD", spec, subdim=False, uops_sha={})
    dve_ops.OPS.append(op)
    dve_ops.CUSTOM_DVE_SPECS[op.name] = op.spec
    dve_ops._SUB_OPCODE_FOR_NAME[op.name] = (
        dve_ops._CUSTOM_DVE_ROW_BASE + len(dve_ops.OPS) - 1
    )
    ver = dve_ver_for("TRN2")
    try:
        op.compile(ver)
    except ValueError as e:
        m = re.search(r'uops_sha\["%s"\]="([0-9a-f]+)"' % ver, str(e))
        if not m:
            raise
        op.uops_sha[ver] = m.group(1)
        op.compile(ver)
    return op


def _build():
    MEMUPD = _register_memupd()
    nc = Bacc()
    f32 = mybir.dt.float32
    bf16 = mybir.dt.bfloat16
    AF = mybir.ActivationFunctionType

    xT_d = nc.declare_dram_parameter("xT", [KT, 128, BC], f32, isOutput=False)
    w1T_d = nc.declare_dram_parameter("w1T", [KT, 128, D_H_PAD], f32, isOutput=False)
    w2p_d = nc.declare_dram_parameter("w2p", [128, HT, MW], bf16, isOutput=False)
    evT_d = nc.declare_dram_parameter("evT", [NQ, NE, 128, 2 * QW], f32, isOutput=True)

    with tile.TileContext(nc) as tc, ExitStack() as ctx:
        pool = ctx.enter_context(tc.tile_pool(name="sb", bufs=1))
        ppool = ctx.enter_context(tc.tile_pool(name="ps", bufs=1, space="PSUM"))

        xsb = [pool.tile([128, KT, QW], f32, name=f"x_{i}") for i in range(2)]
        w1sb = pool.tile([128, KT, D_H_PAD], f32)
        cur1q = [pool.tile([128, HT, QW], f32, name=f"cur1_{i}") for i in range(2)]
        mem = [pool.tile([128, HT, QW], f32, name=f"mem_{i}") for i in range(6)]
        # sgn holds a PAIR of timesteps side by side so fc2 runs N=512 matmuls
        sgn = [pool.tile([128, HT, 2, QW], bf16, name=f"sgn_{i}") for i in range(3)]
        w2sb = pool.tile([128, HT, MW], bf16)
        evsb = [pool.tile([128, 2 * QW], f32, name=f"ev_{i}") for i in range(2)]
        negone = pool.tile([128, 1], f32)
        zero = pool.tile([128, 1], f32)

        # fc1 psum: 4 h-tiles of one quarter per tile (2 banks), ping-pong
        p1 = [ppool.tile([128, 4, QW], f32, name=f"p1_{i}") for i in range(2)]
        # fc2 psum: 4 step-PAIRS packed at partition col-groups 32*j (8 steps)
        p2 = [ppool.tile([128, 2 * QW], f32, name=f"p2_{i}") for i in range(2)]

        nc.gpsimd.memset(negone[:], -1.0)
        nc.gpsimd.memset(zero[:], 0.0)

        # Input DMAs, ordered so fc1(q0) can start as early as possible.
        # x streams through 2 rotating quarter buffers; x(q) is DMA'd during
        # quarter q-1 (emitted with the drip).
        nc.sync.dma_start(w2sb[:], w2p_d[:])
        for k in range(KT):
            nc.sync.dma_start(w1sb[:, k, 0:512], w1T_d[k, :, 0:512])
            nc.sync.dma_start(xsb[0][:, k, :], xT_d[k, :, 0:QW])
        for k in range(KT):
            nc.sync.dma_start(w1sb[:, k, 512:1024], w1T_d[k, :, 512:1024])

        def fc1_thunks(q, copies_on_vector=False):
            """fc1 for quarter q: 56 mm thunks, then the 2 copy thunks last."""
            qs = slice(q * QW, (q + 1) * QW)
            xq = xsb[q % 2]
            mms = []
            cps = []
            for g in range(2):
                par = (2 * q + g) % 2

                for h4 in range(4):
                    h = 4 * g + h4
                    for k in range(KT):
                        def mm(h=h, h4=h4, k=k, par=par):
                            nc.tensor.matmul(
                                p1[par][:, h4, :],
                                w1sb[:, k, 128 * h : 128 * (h + 1)],
                                xq[:, k, :],
                                start=(k == 0),
                                stop=(k == KT - 1),
                            )
                        mms.append(mm)

                def cp(g=g, par=par):
                    dst = cur1q[q % 2][:, 4 * g : 4 * (g + 1), :]
                    if copies_on_vector:
                        nc.vector.tensor_copy(dst, p1[par][:])
                    else:
                        nc.scalar.activation(dst, p1[par][:], AF.Identity, bias=zero[:])
                cps.append(cp)
            # interleave: [g0 mms, cp_g0, g1 mms, cp_g1]
            return mms[:28] + [cps[0]] + mms[28:] + [cps[1]], cps

        # Head: fc1(q0) with g0/g1 interleaved into the vector/scalar queues so
        # the first memupd+sign can start after only the first 4 h-tiles.
        q0thunks, _ = fc1_thunks(0, copies_on_vector=True)
        qs0 = slice(0, QW)
        for th in q0thunks[:29]:  # g0 mms + cur1-g0 copy (vector)
            th()
        for th in q0thunks[29:57]:  # g1 mms
            th()
        g0, g1 = slice(0, 4), slice(4, HT)
        nc.scalar.activation(
            sgn[0][:, g0, 0, :], cur1q[0][:, g0, :], AF.Sign, bias=negone[:]
        )
        nc.vector._custom_dve(
            MEMUPD,
            out=mem[1][:, g0, :],
            in0=cur1q[0][:, g0, :],
            in1=cur1q[0][:, g0, :],
            s0=BETA,
            s1=THR,
        )
        q0thunks[57]()  # vector: cur1 g1 copy
        nc.scalar.activation(
            sgn[0][:, g1, 0, :], cur1q[0][:, g1, :], AF.Sign, bias=negone[:]
        )
        nc.vector._custom_dve(
            MEMUPD,
            out=mem[1][:, g1, :],
            in0=cur1q[0][:, g1, :],
            in1=cur1q[0][:, g1, :],
            s0=BETA,
            s1=THR,
        )
        for k in range(KT):
            nc.sync.dma_start(xsb[1][:, k, :], xT_d[k, :, QW : 2 * QW])

        pending_ev = [None]

        def flush_ev():
            if pending_ev[0] is not None:
                pending_ev[0]()
                pending_ev[0] = None

        for q in range(NQ):
            qs = slice(q * QW, (q + 1) * QW)
            if q + 1 < NQ:
                drip, _ = fc1_thunks(q + 1)
                if q + 2 < NQ:
                    qn = q + 2

                    def xdma(qn=qn):
                        for k in range(KT):
                            nc.sync.dma_start(
                                xsb[qn % 2][:, k, :],
                                xT_d[k, :, qn * QW : (qn + 1) * QW],
                            )
                    drip = drip + [xdma]
            else:
                drip = []
            emitted = 0
            for t in range(T):
                par = t % 6
                cq = cur1q[q % 2]
                if t > 0 and not (q == 0 and t == 1):
                    # split t=1 per h-group so it can start as soon as the
                    # first half of cur1(q) lands
                    for g in [g0, g1] if t == 1 else [slice(0, HT)]:
                        in0 = (
                            cq[:, g, :]
                            if t == 1
                            else mem[(t - 1) % 6][:, g, :]
                        )
                        nc.vector._custom_dve(
                            MEMUPD,
                            out=mem[par][:, g, :],
                            in0=in0,
                            in1=cq[:, g, :],
                            s0=BETA,
                            s1=THR,
                        )
                sp = (t // 2) % 3
                if not (q == 0 and t == 0):
                    msrc = cq[:, :, :] if t == 0 else mem[par][:, :, :]
                    nc.scalar.activation(
                        sgn[sp][:, :, t % 2, :], msrc, AF.Sign, bias=negone[:]
                    )
                if t % 8 == 3:
                    flush_ev()

                # fc2 on step-pairs (and the lone final step)
                if t % 2 == 1 or t == T - 1:
                    j = (t // 2) % 4
                    epar = (t // 8) % 2
                    pair = t % 2 == 1
                    for h in range(HT):
                        rhs = (
                            sgn[sp][:, h, :, :] if pair else sgn[sp][:, h, 0, :]
                        )
                        out_ap = (
                            p2[epar][32 * j : 32 * j + MW, :]
                            if pair
                            else p2[epar][32 * j : 32 * j + MW, 0:QW]
                        )
                        nc.tensor.matmul(
                            out_ap,
                            w2sb[:, h, :],
                            rhs,
                            start=(h == 0),
                            stop=(h == HT - 1),
                            tile_position=(0, 32 * j),
                        )

                if drip:
                    target = min(len(drip), (t + 1) * len(drip) // 21 + 1)
                    while emitted < target:
                        drip[emitted]()
                        emitted += 1

                if t % 8 == 7 or t == T - 1:
                    e = t // 8
                    evp = e % 2
                    ep = epar

                    def ev(q=q, e=e, evp=evp, ep=ep):
                        nc.scalar.activation(
                            evsb[evp][:], p2[ep][:], AF.Identity, bias=zero[:]
                        )
                        nc.sync.dma_start(evT_d[q, e, :, :], evsb[evp][:])

                    flush_ev()
                    pending_ev[0] = ev
        flush_ev()

    nc.finalize()
    return nc


def _prep_shared(W1, b1, W2, b2):
    bf = ml_dtypes.bfloat16
    w1T = np.zeros((KT * 128, D_H_PAD), np.float32)
    w1T[:D_IN, :D_H] = W1.T
    w1T[D_IN, :D_H] = b1  # ones-row trick: bias folded into the contraction
    w1T = np.ascontiguousarray(w1T.reshape(KT, 128, D_H_PAD))

    w2pad = np.zeros((D_OUT, D_H_PAD), np.float32)
    w2pad[:, :D_H] = W2
    terms = []
    r = w2pad.copy()
    for _ in range(NTERMS):
        tb = r.astype(bf)
        terms.append(tb)
        r = r - tb.astype(np.float32)

    w2p = np.zeros((128, HT, MW), bf)
    for h in range(HT):
        for ti, tb in enumerate(terms):
            half = (0.5 * tb[:, 128 * h : 128 * (h + 1)].astype(np.float32)).astype(bf)
            w2p[:, h, D_OUT * ti : D_OUT * (ti + 1)] = half.T

    # spk = (sgn+1)/2 so W2@spk = 0.5*W2@sgn + 0.5*sum(W2); fold shift into v.
    v = (b2.astype(np.float64) + 0.5 * w2pad.astype(np.float64).sum(axis=1)).astype(
        np.float32
    )
    return w1T, w2p, v


def kernel(**inputs):
    global LAST_EXEC_NS
    x = np.ascontiguousarray(np.asarray(inputs["x"], dtype=np.float32))
    W1 = np.asarray(inputs["W1"], dtype=np.float32)
    b1 = np.asarray(inputs["b1"], dtype=np.float32)
    W2 = np.asarray(inputs["W2"], dtype=np.float32)
    b2 = np.asarray(inputs["b2"], dtype=np.float32)

    if "nc" not in _CACHE:
        _CACHE["nc"] = _build()
    nc = _CACHE["nc"]

    w1T, w2p, v = _prep_shared(W1, b1, W2, b2)

    in_maps = []
    for c in range(N_CORES):
        xc = x[c * BC : (c + 1) * BC]  # [BC, 784]
        xT = np.zeros((KT * 128, BC), np.float32)
        xT[:D_IN] = xc.T
        xT[D_IN] = 1.0  # ones-row carries b1
        in_maps.append(
            {
                "xT": np.ascontiguousarray(xT.reshape(KT, 128, BC)),
                "w1T": w1T,
                "w2p": w2p,
            }
        )

    if TRACE:
        _install_ntff_hook()
    br = run_bass_kernel_spmd(nc, in_maps, list(range(N_CORES)), trace=TRACE)
    LAST_EXEC_NS = br.exec_time_ns

    # cur2 = 0.5*W2@sgn + v, summed over the 3 bf16 terms on host.
    X = np.empty((T, B, D_OUT), np.float32)
    for c in range(N_CORES):
        evT = br.results[c]["evT"]  # [NQ, NE, 128, 2*QW]
        for q in range(NQ):
            for e in range(NE):
                for j in range(4):
                    for s in range(2):
                        t = 8 * e + 2 * j + s
                        if t >= T:
                            break
                        rows = evT[
                            q, e, 32 * j : 32 * j + MW, s * QW : (s + 1) * QW
                        ]  # [30, QW]
                        ssum = (
                            rows[0:D_OUT]
                            + rows[D_OUT : 2 * D_OUT]
                            + rows[2 * D_OUT :]
                        )
                        b0 = c * BC + q * QW
                        X[t, b0 : b0 + QW, :] = ssum.T

    X += v

    mem2_rec = np.empty((T, B, D_OUT), np.float32)
    beta = np.float32(BETA)
    m2 = X[0]
    mem2_rec[0] = m2
    for t in range(1, T):
        spk = (m2 > THR).astype(np.float32)
        m2 = beta * m2 + X[t] - spk
        mem2_rec[t] = m2
    spk2_rec = (mem2_rec > THR).astype(np.float32)
    return spk2_rec, mem2_rec


# revision 26
# speedup vs baseline: 1.1856x; 1.0324x over previous
import re
import sys
from contextlib import ExitStack

import numpy as np

try:
    import concourse  # noqa
except ImportError:
    sys.path.insert(0, "/opt/trn_rl_repo")

import ml_dtypes
import concourse.bass as bass  # noqa
import concourse.dve_ops as dve_ops
import concourse.tile as tile
from concourse import mybir
from concourse.bass_utils import run_bass_kernel_spmd
from concourse.dve_ops import DveOp
from concourse.dve_spec import C0, C1, Spec, Src0, Src1
from concourse.dve_table_gen import dve_ver_for
from concourse.bacc import Bacc

N_CORES = 8
B = 8192
BC = B // N_CORES  # 1024 batch per core
D_IN = 784
KT = 7  # 784 -> 7 k-tiles of 128 (row 784 = ones carries the bias)
D_IN_PAD = KT * 128  # 896
D_H = 1000
HT = 8  # 1000 -> 8 h-tiles of 128
D_H_PAD = HT * 128  # 1024
D_OUT = 10
T = 25
BETA = 0.95
THR = 1.0
NQ = 4  # batch quarters per core
QW = BC // NQ  # 256
NTERMS = 3  # exact bf16 decomposition of W2
MW = NTERMS * D_OUT  # 30: term ti at columns 10*ti..10*ti+9
NE = (T + 7) // 8  # 4 blocks of up-to-8 timesteps packed into one psum tile

LAST_EXEC_NS = None
TRACE = False

_CACHE = {}


def _install_ntff_hook():
    try:
        import antenv.axon_hooks  # noqa

        return
    except ImportError:
        pass
    try:
        import types

        import antenv

        mod = types.ModuleType("antenv.axon_hooks")
        mod._hook = None

        def set_axon_ntff_profile_hook(h):
            mod._hook = h

        def get_axon_ntff_profile_hook():
            return mod._hook

        mod.set_axon_ntff_profile_hook = set_axon_ntff_profile_hook
        mod.get_axon_ntff_profile_hook = get_axon_ntff_profile_hook
        sys.modules["antenv.axon_hooks"] = mod
        antenv.axon_hooks = mod
        try:
            from trn_agent_boot.trn_boot import _ntff_profile_via_ctypes

            hook = _ntff_profile_via_ctypes("/opt/axon/libaxon_pjrt.so")
            if hook is not None:
                mod._hook = hook
        except Exception:
            pass
    except Exception:
        pass


def _register_memupd():
    for op in dve_ops.OPS:
        if op.name == "SNN_MEMUPD":
            return op
    spec = Spec(
        body=Src0 * C0 + Src1 - (Src0 > C1),
        reference=lambda in0, in1, s0, s1, imm2: in0 * s0
        + in1
        - (in0 > s1).astype(np.float32),
    )
    op = DveOp("SNN_MEMUPD", spec, subdim=False, uops_sha={})
    dve_ops.OPS.append(op)
    dve_ops.CUSTOM_DVE_SPECS[op.name] = op.spec
    dve_ops._SUB_OPCODE_FOR_NAME[op.name] = (
        dve_ops._CUSTOM_DVE_ROW_BASE + len(dve_ops.OPS) - 1
    )
    ver = dve_ver_for("TRN2")
    try:
        op.compile(ver)
    except ValueError as e:
        m = re.search(r'uops_sha\["%s"\]="([0-9a-f]+)"' % ver, str(e))
        if not m:
            raise
        op.uops_sha[ver] = m.group(1)
        op.compile(ver)
    return op


def _build():
    MEMUPD = _register_memupd()
    nc = Bacc()
    f32 = mybir.dt.float32
    bf16 = mybir.dt.bfloat16
    AF = mybir.ActivationFunctionType

    xT_d = nc.declare_dram_parameter("xT", [KT, 128, BC], f32, isOutput=False)
    w1T_d = nc.declare_dram_parameter("w1T", [KT, 128, D_H_PAD], f32, isOutput=False)
    w2p_d = nc.declare_dram_parameter("w2p", [128, HT, MW], bf16, isOutput=False)
    evT_d = nc.declare_dram_parameter("evT", [NQ, NE, 128, 2 * QW], f32, isOutput=True)

    with tile.TileContext(nc) as tc, ExitStack() as ctx:
        pool = ctx.enter_context(tc.tile_pool(name="sb", bufs=1))
        ppool = ctx.enter_context(tc.tile_pool(name="ps", bufs=1, space="PSUM"))

        xsb = [pool.tile([128, KT, QW], f32, name=f"x_{i}") for i in range(2)]
        w1sb = pool.tile([128, KT, D_H_PAD], f32)
        cur1q = [pool.tile([128, HT, QW], f32, name=f"cur1_{i}") for i in range(2)]
        mem = [pool.tile([128, HT, QW], f32, name=f"mem_{i}") for i in range(6)]
        # sgn holds a PAIR of timesteps side by side so fc2 runs N=512 matmuls
        sgn = [pool.tile([128, HT, 2, QW], bf16, name=f"sgn_{i}") for i in range(4)]
        w2sb = pool.tile([128, HT, MW], bf16)
        evsb = [pool.tile([128, 2 * QW], f32, name=f"ev_{i}") for i in range(2)]
        negone = pool.tile([128, 1], f32)
        zero = pool.tile([128, 1], f32)

        # fc1 psum: 4 h-tiles of one quarter per tile (2 banks), ping-pong
        p1 = [ppool.tile([128, 4, QW], f32, name=f"p1_{i}") for i in range(2)]
        # fc2 psum: 4 step-PAIRS packed at partition col-groups 32*j (8 steps)
        p2 = [ppool.tile([128, 2 * QW], f32, name=f"p2_{i}") for i in range(2)]

        nc.gpsimd.memset(negone[:], -1.0)
        nc.gpsimd.memset(zero[:], 0.0)

        # Input DMAs, ordered so fc1(q0) can start as early as possible.
        # x streams through 2 rotating quarter buffers; x(q) is DMA'd during
        # quarter q-1 (emitted with the drip).
        for k in range(KT):
            nc.sync.dma_start(w1sb[:, k, 0:512], w1T_d[k, :, 0:512])
            nc.sync.dma_start(xsb[0][:, k, :], xT_d[k, :, 0:QW])
        nc.sync.dma_start(w2sb[:], w2p_d[:])
        for k in range(KT):
            nc.sync.dma_start(w1sb[:, k, 512:1024], w1T_d[k, :, 512:1024])

        def fc1_thunks(q, copies_on_vector=False):
            """fc1 for quarter q: 56 mm thunks, then the 2 copy thunks last."""
            qs = slice(q * QW, (q + 1) * QW)
            xq = xsb[q % 2]
            mms = []
            cps = []
            for g in range(2):
                par = (2 * q + g) % 2

                for h4 in range(4):
                    h = 4 * g + h4
                    for k in range(KT):
                        def mm(h=h, h4=h4, k=k, par=par):
                            nc.tensor.matmul(
                                p1[par][:, h4, :],
                                w1sb[:, k, 128 * h : 128 * (h + 1)],
                                xq[:, k, :],
                                start=(k == 0),
                                stop=(k == KT - 1),
                            )
                        mms.append(mm)

                def cp(g=g, par=par):
                    dst = cur1q[q % 2][:, 4 * g : 4 * (g + 1), :]
                    if copies_on_vector:
                        nc.vector.tensor_copy(dst, p1[par][:])
                    else:
                        nc.scalar.activation(dst, p1[par][:], AF.Identity, bias=zero[:])
                cps.append(cp)
            # interleave: [g0 mms, cp_g0, g1 mms, cp_g1]
            return mms[:28] + [cps[0]] + mms[28:] + [cps[1]], cps

        # Head: fc1(q0) with g0/g1 interleaved into the vector/scalar queues so
        # the first memupd+sign can start after only the first 4 h-tiles; g0
        # then runs GA steps ahead while fc1(q0,g1) finishes on the PE.
        q0thunks, _ = fc1_thunks(0, copies_on_vector=True)
        for th in q0thunks[:29]:  # g0 mms + cur1-g0 copy (vector)
            th()
        for th in q0thunks[29:57]:  # g1 mms
            th()
        g0, g1 = slice(0, 4), slice(4, HT)

        def memupd_g(t, g, cq):
            in0 = cq[:, g, :] if t == 1 else mem[(t - 1) % 6][:, g, :]
            nc.vector._custom_dve(
                MEMUPD,
                out=mem[t % 6][:, g, :],
                in0=in0,
                in1=cq[:, g, :],
                s0=BETA,
                s1=THR,
            )

        def sign_g(t, g, cq):
            msrc = cq[:, g, :] if t == 0 else mem[t % 6][:, g, :]
            nc.scalar.activation(
                sgn[(t // 2) % 4][:, g, t % 2, :], msrc, AF.Sign, bias=negone[:]
            )

        def fc2_emit(q, t):
            sp = (t // 2) % 4
            j = (t // 2) % 4
            epar = (t // 8) % 2
            pair = t % 2 == 1
            for h in range(HT):
                rhs = sgn[sp][:, h, :, :] if pair else sgn[sp][:, h, 0, :]
                out_ap = (
                    p2[epar][32 * j : 32 * j + MW, :]
                    if pair
                    else p2[epar][32 * j : 32 * j + MW, 0:QW]
                )
                nc.tensor.matmul(
                    out_ap,
                    w2sb[:, h, :],
                    rhs,
                    start=(h == 0),
                    stop=(h == HT - 1),
                    tile_position=(0, 32 * j),
                )

        pending_ev = [None]

        def flush_ev():
            if pending_ev[0] is not None:
                pending_ev[0]()
                pending_ev[0] = None

        GA = 7
        cq0 = cur1q[0]
        sign_g(0, g0, cq0)
        for t in range(1, GA + 1):
            memupd_g(t, g0, cq0)
            sign_g(t, g0, cq0)
        q0thunks[57]()  # vector: cur1 g1 copy
        sign_g(0, g1, cq0)
        for t in range(1, GA + 1):
            memupd_g(t, g1, cq0)
            sign_g(t, g1, cq0)
            if t % 2 == 1:
                fc2_emit(0, t)
        for k in range(KT):
            nc.sync.dma_start(xsb[1][:, k, :], xT_d[k, :, QW : 2 * QW])

        def ev0():
            nc.scalar.activation(evsb[0][:], p2[0][:], AF.Identity, bias=zero[:])
            nc.sync.dma_start(evT_d[0, 0, :, :], evsb[0][:])

        pending_ev[0] = ev0

        for q in range(NQ):
            qs = slice(q * QW, (q + 1) * QW)
            if q + 1 < NQ:
                drip, _ = fc1_thunks(q + 1)
                if q + 2 < NQ:
                    qn = q + 2

                    def xdma(qn=qn):
                        for k in range(KT):
                            nc.sync.dma_start(
                                xsb[qn % 2][:, k, :],
                                xT_d[k, :, qn * QW : (qn + 1) * QW],
                            )
                    drip = drip + [xdma]
            else:
                drip = []
            emitted = 0
            for t in range(8 if q == 0 else 0, T):
                par = t % 6
                cq = cur1q[q % 2]
                if t > 0:
                    in0 = cq[:, :, :] if t == 1 else mem[(t - 1) % 6][:, :, :]
                    nc.vector._custom_dve(
                        MEMUPD,
                        out=mem[par][:, :, :],
                        in0=in0,
                        in1=cq[:, :, :],
                        s0=BETA,
                        s1=THR,
                    )
                nc.scalar.activation(
                    sgn[(t // 2) % 4][:, :, t % 2, :],
                    cq[:, :, :] if t == 0 else mem[par][:, :, :],
                    AF.Sign,
                    bias=negone[:],
                )
                if t % 8 == 3:
                    flush_ev()

                # fc2 on step-pairs (and the lone final step)
                if t % 2 == 1 or t == T - 1:
                    fc2_emit(q, t)

                if drip:
                    target = min(len(drip), (t + 1) * len(drip) // 21 + 1)
                    while emitted < target:
                        drip[emitted]()
                        emitted += 1

                if t % 8 == 7 or t == T - 1:
                    e = t // 8
                    evp = e % 2
                    ep = (t // 8) % 2

                    def ev(q=q, e=e, evp=evp, ep=ep):
                        nc.scalar.activation(
                            evsb[evp][:], p2[ep][:], AF.Identity, bias=zero[:]
                        )
                        nc.sync.dma_start(evT_d[q, e, :, :], evsb[evp][:])

                    flush_ev()
                    pending_ev[0] = ev
        flush_ev()

    nc.finalize()
    return nc


def _prep_shared(W1, b1, W2, b2):
    bf = ml_dtypes.bfloat16
    w1T = np.zeros((KT * 128, D_H_PAD), np.float32)
    w1T[:D_IN, :D_H] = W1.T
    w1T[D_IN, :D_H] = b1  # ones-row trick: bias folded into the contraction
    w1T = np.ascontiguousarray(w1T.reshape(KT, 128, D_H_PAD))

    w2pad = np.zeros((D_OUT, D_H_PAD), np.float32)
    w2pad[:, :D_H] = W2
    terms = []
    r = w2pad.copy()
    for _ in range(NTERMS):
        tb = r.astype(bf)
        terms.append(tb)
        r = r - tb.astype(np.float32)

    w2p = np.zeros((128, HT, MW), bf)
    for h in range(HT):
        for ti, tb in enumerate(terms):
            half = (0.5 * tb[:, 128 * h : 128 * (h + 1)].astype(np.float32)).astype(bf)
            w2p[:, h, D_OUT * ti : D_OUT * (ti + 1)] = half.T

    # spk = (sgn+1)/2 so W2@spk = 0.5*W2@sgn + 0.5*sum(W2); fold shift into v.
    v = (b2.astype(np.float64) + 0.5 * w2pad.astype(np.float64).sum(axis=1)).astype(
        np.float32
    )
    return w1T, w2p, v


def kernel(**inputs):
    global LAST_EXEC_NS
    x = np.ascontiguousarray(np.asarray(inputs["x"], dtype=np.float32))
    W1 = np.asarray(inputs["W1"], dtype=np.float32)
    b1 = np.asarray(inputs["b1"], dtype=np.float32)
    W2 = np.asarray(inputs["W2"], dtype=np.float32)
    b2 = np.asarray(inputs["b2"], dtype=np.float32)

    if "nc" not in _CACHE:
        _CACHE["nc"] = _build()
    nc = _CACHE["nc"]

    w1T, w2p, v = _prep_shared(W1, b1, W2, b2)

    in_maps = []
    for c in range(N_CORES):
        xc = x[c * BC : (c + 1) * BC]  # [BC, 784]
        xT = np.zeros((KT * 128, BC), np.float32)
        xT[:D_IN] = xc.T
        xT[D_IN] = 1.0  # ones-row carries b1
        in_maps.append(
            {
                "xT": np.ascontiguousarray(xT.reshape(KT, 128, BC)),
                "w1T": w1T,
                "w2p": w2p,
            }
        )

    if TRACE:
        _install_ntff_hook()
    br = run_bass_kernel_spmd(nc, in_maps, list(range(N_CORES)), trace=TRACE)
    LAST_EXEC_NS = br.exec_time_ns

    # cur2 = 0.5*W2@sgn + v, summed over the 3 bf16 terms on host.
    X = np.empty((T, B, D_OUT), np.float32)
    for c in range(N_CORES):
        evT = br.results[c]["evT"]  # [NQ, NE, 128, 2*QW]
        for q in range(NQ):
            for e in range(NE):
                for j in range(4):
                    for s in range(2):
                        t = 8 * e + 2 * j + s
                        if t >= T:
                            break
                        rows = evT[
                            q, e, 32 * j : 32 * j + MW, s * QW : (s + 1) * QW
                        ]  # [30, QW]
                        ssum = (
                            rows[0:D_OUT]
                            + rows[D_OUT : 2 * D_OUT]
                            + rows[2 * D_OUT :]
                        )
                        b0 = c * BC + q * QW
                        X[t, b0 : b0 + QW, :] = ssum.T

    X += v

    mem2_rec = np.empty((T, B, D_OUT), np.float32)
    beta = np.float32(BETA)
    m2 = X[0]
    mem2_rec[0] = m2
    for t in range(1, T):
        spk = (m2 > THR).astype(np.float32)
        m2 = beta * m2 + X[t] - spk
        mem2_rec[t] = m2
    spk2_rec = (mem2_rec > THR).astype(np.float32)
    return spk2_rec, mem2_rec


# revision 28
# speedup vs baseline: 1.1862x; 1.0005x over previous
import re
import sys
from contextlib import ExitStack

import numpy as np

try:
    import concourse  # noqa
except ImportError:
    sys.path.insert(0, "/opt/trn_rl_repo")

import ml_dtypes
import concourse.bass as bass  # noqa
import concourse.dve_ops as dve_ops
import concourse.tile as tile
from concourse import mybir
from concourse.bass_utils import run_bass_kernel_spmd
from concourse.dve_ops import DveOp
from concourse.dve_spec import C0, C1, Spec, Src0, Src1
from concourse.dve_table_gen import dve_ver_for
from concourse.bacc import Bacc

N_CORES = 8
B = 8192
BC = B // N_CORES  # 1024 batch per core
D_IN = 784
KT = 7  # 784 -> 7 k-tiles of 128 (row 784 = ones carries the bias)
D_IN_PAD = KT * 128  # 896
D_H = 1000
HT = 8  # 1000 -> 8 h-tiles of 128
D_H_PAD = HT * 128  # 1024
D_OUT = 10
T = 25
BETA = 0.95
THR = 1.0
NQ = 4  # batch quarters per core
QW = BC // NQ  # 256
NTERMS = 3  # exact bf16 decomposition of W2
MW = NTERMS * D_OUT  # 30: term ti at columns 10*ti..10*ti+9
NE = (T + 7) // 8  # 4 blocks of up-to-8 timesteps packed into one psum tile

LAST_EXEC_NS = None
TRACE = False

_CACHE = {}


def _install_ntff_hook():
    try:
        import antenv.axon_hooks  # noqa

        return
    except ImportError:
        pass
    try:
        import types

        import antenv

        mod = types.ModuleType("antenv.axon_hooks")
        mod._hook = None

        def set_axon_ntff_profile_hook(h):
            mod._hook = h

        def get_axon_ntff_profile_hook():
            return mod._hook

        mod.set_axon_ntff_profile_hook = set_axon_ntff_profile_hook
        mod.get_axon_ntff_profile_hook = get_axon_ntff_profile_hook
        sys.modules["antenv.axon_hooks"] = mod
        antenv.axon_hooks = mod
        try:
            from trn_agent_boot.trn_boot import _ntff_profile_via_ctypes

            hook = _ntff_profile_via_ctypes("/opt/axon/libaxon_pjrt.so")
            if hook is not None:
                mod._hook = hook
        except Exception:
            pass
    except Exception:
        pass


def _register_memupd():
    for op in dve_ops.OPS:
        if op.name == "SNN_MEMUPD":
            return op
    spec = Spec(
        body=Src0 * C0 + Src1 - (Src0 > C1),
        reference=lambda in0, in1, s0, s1, imm2: in0 * s0
        + in1
        - (in0 > s1).astype(np.float32),
    )
    op = DveOp("SNN_MEMUPD", spec, subdim=False, uops_sha={})
    dve_ops.OPS.append(op)
    dve_ops.CUSTOM_DVE_SPECS[op.name] = op.spec
    dve_ops._SUB_OPCODE_FOR_NAME[op.name] = (
        dve_ops._CUSTOM_DVE_ROW_BASE + len(dve_ops.OPS) - 1
    )
    ver = dve_ver_for("TRN2")
    try:
        op.compile(ver)
    except ValueError as e:
        m = re.search(r'uops_sha\["%s"\]="([0-9a-f]+)"' % ver, str(e))
        if not m:
            raise
        op.uops_sha[ver] = m.group(1)
        op.compile(ver)
    return op


def _build():
    MEMUPD = _register_memupd()
    nc = Bacc()
    f32 = mybir.dt.float32
    bf16 = mybir.dt.bfloat16
    AF = mybir.ActivationFunctionType

    xT_d = nc.declare_dram_parameter("xT", [KT, 128, BC], f32, isOutput=False)
    w1T_d = nc.declare_dram_parameter("w1T", [KT, 128, D_H_PAD], f32, isOutput=False)
    w2p_d = nc.declare_dram_parameter("w2p", [128, HT, MW], bf16, isOutput=False)
    evT_d = nc.declare_dram_parameter("evT", [NQ, NE, 128, 2 * QW], f32, isOutput=True)

    with tile.TileContext(nc) as tc, ExitStack() as ctx:
        pool = ctx.enter_context(tc.tile_pool(name="sb", bufs=1))
        ppool = ctx.enter_context(tc.tile_pool(name="ps", bufs=1, space="PSUM"))

        xsb = [pool.tile([128, KT, QW], f32, name=f"x_{i}") for i in range(2)]
        w1sb = pool.tile([128, KT, D_H_PAD], f32)
        cur1q = [pool.tile([128, HT, QW], f32, name=f"cur1_{i}") for i in range(2)]
        mem = [pool.tile([128, HT, QW], f32, name=f"mem_{i}") for i in range(6)]
        # sgn holds a PAIR of timesteps side by side so fc2 runs N=512 matmuls
        sgn = [pool.tile([128, HT, 2, QW], bf16, name=f"sgn_{i}") for i in range(4)]
        w2sb = pool.tile([128, HT, MW], bf16)
        evsb = [pool.tile([128, 2 * QW], f32, name=f"ev_{i}") for i in range(2)]
        negone = pool.tile([128, 1], f32)
        zero = pool.tile([128, 1], f32)

        # fc1 psum: 4 h-tiles of one quarter per tile (2 banks), ping-pong
        p1 = [ppool.tile([128, 4, QW], f32, name=f"p1_{i}") for i in range(2)]
        # fc2 psum: 4 step-PAIRS packed at partition col-groups 32*j (8 steps)
        p2 = [ppool.tile([128, 2 * QW], f32, name=f"p2_{i}") for i in range(2)]

        nc.gpsimd.memset(negone[:], -1.0)
        nc.gpsimd.memset(zero[:], 0.0)

        # Input DMAs, ordered so fc1(q0) can start as early as possible.
        # x streams through 2 rotating quarter buffers; x(q) is DMA'd during
        # quarter q-1 (emitted with the drip).
        for k in range(KT):
            nc.sync.dma_start(w1sb[:, k, 0:512], w1T_d[k, :, 0:512])
            nc.sync.dma_start(xsb[0][:, k, :], xT_d[k, :, 0:QW])
        nc.sync.dma_start(w2sb[:], w2p_d[:])
        for k in range(KT):
            nc.sync.dma_start(w1sb[:, k, 512:1024], w1T_d[k, :, 512:1024])

        def fc1_thunks(q, copies_on_vector=False):
            """fc1 for quarter q: 56 mm thunks, then the 2 copy thunks last."""
            qs = slice(q * QW, (q + 1) * QW)
            xq = xsb[q % 2]
            mms = []
            cps = []
            for g in range(2):
                par = (2 * q + g) % 2

                for h4 in range(4):
                    h = 4 * g + h4
                    for k in range(KT):
                        def mm(h=h, h4=h4, k=k, par=par):
                            nc.tensor.matmul(
                                p1[par][:, h4, :],
                                w1sb[:, k, 128 * h : 128 * (h + 1)],
                                xq[:, k, :],
                                start=(k == 0),
                                stop=(k == KT - 1),
                            )
                        mms.append(mm)

                def cp(g=g, par=par):
                    dst = cur1q[q % 2][:, 4 * g : 4 * (g + 1), :]
                    if copies_on_vector:
                        nc.vector.tensor_copy(dst, p1[par][:])
                    else:
                        nc.scalar.activation(dst, p1[par][:], AF.Identity, bias=zero[:])
                cps.append(cp)
            # interleave: [g0 mms, cp_g0, g1 mms, cp_g1]
            return mms[:28] + [cps[0]] + mms[28:] + [cps[1]], cps

        # Head: fc1(q0) with g0/g1 interleaved into the vector/scalar queues so
        # the first memupd+sign can start after only the first 4 h-tiles; g0
        # then runs GA steps ahead while fc1(q0,g1) finishes on the PE.
        q0thunks, _ = fc1_thunks(0, copies_on_vector=True)
        for th in q0thunks[:29]:  # g0 mms + cur1-g0 copy (vector)
            th()
        for th in q0thunks[29:57]:  # g1 mms
            th()
        g0, g1 = slice(0, 4), slice(4, HT)

        def memupd_g(t, g, cq):
            in0 = cq[:, g, :] if t == 1 else mem[(t - 1) % 6][:, g, :]
            nc.vector._custom_dve(
                MEMUPD,
                out=mem[t % 6][:, g, :],
                in0=in0,
                in1=cq[:, g, :],
                s0=BETA,
                s1=THR,
            )

        def sign_g(t, g, cq):
            msrc = cq[:, g, :] if t == 0 else mem[t % 6][:, g, :]
            nc.scalar.activation(
                sgn[(t // 2) % 4][:, g, t % 2, :], msrc, AF.Sign, bias=negone[:]
            )

        def fc2_emit(q, t):
            sp = (t // 2) % 4
            j = (t // 2) % 4
            epar = (t // 8) % 2
            pair = t % 2 == 1
            for h in range(HT):
                rhs = sgn[sp][:, h, :, :] if pair else sgn[sp][:, h, 0, :]
                out_ap = (
                    p2[epar][32 * j : 32 * j + MW, :]
                    if pair
                    else p2[epar][32 * j : 32 * j + MW, 0:QW]
                )
                nc.tensor.matmul(
                    out_ap,
                    w2sb[:, h, :],
                    rhs,
                    start=(h == 0),
                    stop=(h == HT - 1),
                    tile_position=(0, 32 * j),
                )

        pending_ev = [None]

        def flush_ev():
            if pending_ev[0] is not None:
                pending_ev[0]()
                pending_ev[0] = None

        GA = 7
        cq0 = cur1q[0]
        sign_g(0, g0, cq0)
        for t in range(1, GA + 1):
            memupd_g(t, g0, cq0)
            sign_g(t, g0, cq0)
        q0thunks[57]()  # vector: cur1 g1 copy
        sign_g(0, g1, cq0)
        for t in range(1, GA + 1):
            memupd_g(t, g1, cq0)
            sign_g(t, g1, cq0)
            if t % 2 == 1:
                fc2_emit(0, t)
        for k in range(KT):
            nc.sync.dma_start(xsb[1][:, k, :], xT_d[k, :, QW : 2 * QW])

        def ev0():
            nc.scalar.activation(evsb[0][:], p2[0][:], AF.Identity, bias=zero[:])
            nc.sync.dma_start(evT_d[0, 0, :, :], evsb[0][:])

        pending_ev[0] = ev0

        for q in range(NQ):
            qs = slice(q * QW, (q + 1) * QW)
            if q + 1 < NQ:
                drip, _ = fc1_thunks(q + 1)
                if q + 2 < NQ:
                    qn = q + 2

                    def xdma(qn=qn):
                        for k in range(KT):
                            nc.sync.dma_start(
                                xsb[qn % 2][:, k, :],
                                xT_d[k, :, qn * QW : (qn + 1) * QW],
                            )
                    drip = drip + [xdma]
            else:
                drip = []
            emitted = 0
            for t in range(8 if q == 0 else 0, T):
                par = t % 6
                cq = cur1q[q % 2]
                if t > 0:
                    in0 = cq[:, :, :] if t == 1 else mem[(t - 1) % 6][:, :, :]
                    nc.vector._custom_dve(
                        MEMUPD,
                        out=mem[par][:, :, :],
                        in0=in0,
                        in1=cq[:, :, :],
                        s0=BETA,
                        s1=THR,
                    )
                nc.scalar.activation(
                    sgn[(t // 2) % 4][:, :, t % 2, :],
                    cq[:, :, :] if t == 0 else mem[par][:, :, :],
                    AF.Sign,
                    bias=negone[:],
                )
                if t % 8 == 3:
                    flush_ev()

                # fc2 on step-pairs (and the lone final step)
                if t % 2 == 1 or t == T - 1:
                    fc2_emit(q, t)

                if drip:
                    target = min(len(drip), (t + 1) * len(drip) // 21 + 1)
                    while emitted < target:
                        drip[emitted]()
                        emitted += 1

                if t % 8 == 7 or t == T - 1:
                    e = t // 8
                    evp = e % 2
                    ep = (t // 8) % 2

                    def ev(q=q, e=e, evp=evp, ep=ep):
                        nc.scalar.activation(
                            evsb[evp][:], p2[ep][:], AF.Identity, bias=zero[:]
                        )
                        nc.sync.dma_start(evT_d[q, e, :, :], evsb[evp][:])

                    flush_ev()
                    pending_ev[0] = ev
        flush_ev()

    nc.finalize()
    return nc


def _prep_shared(W1, b1, W2, b2):
    bf = ml_dtypes.bfloat16
    w1T = np.zeros((KT * 128, D_H_PAD), np.float32)
    w1T[:D_IN, :D_H] = W1.T
    w1T[D_IN, :D_H] = b1  # ones-row trick: bias folded into the contraction
    w1T = np.ascontiguousarray(w1T.reshape(KT, 128, D_H_PAD))

    w2pad = np.zeros((D_OUT, D_H_PAD), np.float32)
    w2pad[:, :D_H] = W2
    terms = []
    r = w2pad.copy()
    for _ in range(NTERMS):
        tb = r.astype(bf)
        terms.append(tb)
        r = r - tb.astype(np.float32)

    w2p = np.zeros((128, HT, MW), bf)
    for h in range(HT):
        for ti, tb in enumerate(terms):
            half = (0.5 * tb[:, 128 * h : 128 * (h + 1)].astype(np.float32)).astype(bf)
            w2p[:, h, D_OUT * ti : D_OUT * (ti + 1)] = half.T

    # spk = (sgn+1)/2 so W2@spk = 0.5*W2@sgn + 0.5*sum(W2); fold shift into v.
    v = (b2.astype(np.float64) + 0.5 * w2pad.astype(np.float64).sum(axis=1)).astype(
        np.float32
    )
    return w1T, w2p, v


def kernel(**inputs):
    global LAST_EXEC_NS
    x = np.ascontiguousarray(np.asarray(inputs["x"], dtype=np.float32))
    W1 = np.asarray(inputs["W1"], dtype=np.float32)
    b1 = np.asarray(inputs["b1"], dtype=np.float32)
    W2 = np.asarray(inputs["W2"], dtype=np.float32)
    b2 = np.asarray(inputs["b2"], dtype=np.float32)

    if "nc" not in _CACHE:
        _CACHE["nc"] = _build()
    nc = _CACHE["nc"]

    w1T, w2p, v = _prep_shared(W1, b1, W2, b2)

    in_maps = []
    for c in range(N_CORES):
        xc = x[c * BC : (c + 1) * BC]  # [BC, 784]
        xT = np.zeros((KT * 128, BC), np.float32)
        xT[:D_IN] = xc.T
        xT[D_IN] = 1.0  # ones-row carries b1
        in_maps.append(
            {
                "xT": np.ascontiguousarray(xT.reshape(KT, 128, BC)),
                "w1T": w1T,
                "w2p": w2p,
            }
        )

    if TRACE:
        _install_ntff_hook()
    br = run_bass_kernel_spmd(nc, in_maps, list(range(N_CORES)), trace=TRACE)
    LAST_EXEC_NS = br.exec_time_ns

    # cur2 = 0.5*W2@sgn + v, summed over the 3 bf16 terms on host.
    X = np.empty((T, B, D_OUT), np.float32)
    for c in range(N_CORES):
        evT = br.results[c]["evT"]  # [NQ, NE, 128, 2*QW]
        for q in range(NQ):
            for e in range(NE):
                for j in range(4):
                    for s in range(2):
                        t = 8 * e + 2 * j + s
                        if t >= T:
                            break
                        rows = evT[
                            q, e, 32 * j : 32 * j + MW, s * QW : (s + 1) * QW
                        ]  # [30, QW]
                        ssum = (
                            rows[0:D_OUT]
                            + rows[D_OUT : 2 * D_OUT]
                            + rows[2 * D_OUT :]
                        )
                        b0 = c * BC + q * QW
                        X[t, b0 : b0 + QW, :] = ssum.T

    X += v

    mem2_rec = np.empty((T, B, D_OUT), np.float32)
    beta = np.float32(BETA)
    m2 = X[0]
    mem2_rec[0] = m2
    for t in range(1, T):
        spk = (m2 > THR).astype(np.float32)
        m2 = beta * m2 + X[t] - spk
        mem2_rec[t] = m2
    spk2_rec = (mem2_rec > THR).astype(np.float32)
    return spk2_rec, mem2_rec
